# revision 15
# baseline (speedup 1.0000x reference)
"""AFT local-attention kernel for Trainium2 (8 NeuronCores), v4.

Math (identical to reference in exact arithmetic):
  y = sigmoid(q) * num / den, out = y @ Wo.T + bo, with
    E[i,j] = (exp(pb[i,j]) - 1) * band_mask[i,j]   (band width 127)
    ek     = exp(key @ Wk.T)                       (bk and the exp-max
                                                    stabilizers cancel)
    den    = E @ ek + Z,  Z[b,d] = sum_j ek[j,b,d]  (own rows only)
    num'   = E @ (ek*v') + S', v' = value @ Wv.T    (no bv)
    num/den = num'/den + bv                         (bv folds out)

v4 structure:
  - All inputs host-packed into device-native [128, ...] layouts so every
    DMA line is contiguous per partition (k/v/q split into column groups
    for pipelining).
  - A tiny warmup AllGather at kernel start pre-builds the CC stream so
    the real S/Z pair-exchange has ~8us latency instead of ~26us.
  - Phase order: k/v proj + S/Z -> trigger collective -> q proj (overlaps
    collective) -> einsum/combine/out-proj pipeline (zs2 arrives before
    the first combine needs it).
  - Combine work split across three engines: scalar (den bias-acts + ot
    copy), DVE (num add, reciprocal, y1), Pool (y2 + bv, yts * sig).
  - Output written in bf16; host casts to f32 and adds bo.

Sharding: core c -> batch b=c//2, T-half h=c%2 (2048 queries each); k/v
carry a 64-row halo; S/Z are all-gathered over core pairs [2b, 2b+1].
"""

import numpy as np
from contextlib import ExitStack

import ml_dtypes
import concourse.bass as bass
import concourse.tile as tile
from concourse import bacc, mybir
from concourse.bass import ts, ds
from concourse.bass_utils import run_bass_kernel_spmd

T = 4096
B = 4
D = 512
WIN = 64
TH = T // 2          # tokens per core (query rows)
NIT = TH // 128      # 16 i-tiles
KV = TH + 2 * WIN    # 2176 k/v rows incl. halo
KCH = KV // 128      # 17 k/v chunks

F32 = mybir.dt.float32
BF16 = mybir.dt.bfloat16
FP8 = mybir.dt.float8e4
AF = mybir.ActivationFunctionType
ALU = mybir.AluOpType
DR = mybir.MatmulPerfMode.DoubleRow
BFNP = ml_dtypes.bfloat16
F8NP = ml_dtypes.float8_e4m3
NEG_LN8 = -2.0794415416798357

KV_GROUPS = [(0, 256), (256, 768), (768, 1280), (1280, 1792), (1792, KV)]
QT_GROUPS = [(0, 1024), (1024, TH)]


def build():
    nc = bacc.Bacc("TRN2", target_bir_lowering=False, num_devices=8)

    # flat host-packed inputs; per-partition lines are contiguous
    kt_d = nc.dram_tensor("kt", [128, 4 * KV], BF16, kind="ExternalInput")
    vt_d = nc.dram_tensor("vt", [128, 4 * KV], BF16, kind="ExternalInput")
    qt_d = nc.dram_tensor("qt", [128, 4 * TH], BF16, kind="ExternalInput")
    et_d = nc.dram_tensor("et", [128, 2 * NIT * 128], FP8, kind="ExternalInput")
    wq_d = nc.dram_tensor("wq", [128, 4 * D], BF16, kind="ExternalInput")
    wo_d = nc.dram_tensor("wo", [128, 4 * D], BF16, kind="ExternalInput")
    wk_d = nc.dram_tensor("wk", [128, 4 * D], BF16, kind="ExternalInput")
    wv_d = nc.dram_tensor("wv", [128, 4 * D], BF16, kind="ExternalInput")
    bqt_d = nc.dram_tensor("bqt", [128, 4], F32, kind="ExternalInput")
    bvt_d = nc.dram_tensor("bvt", [128, 4], F32, kind="ExternalInput")
    szm_d = nc.dram_tensor("szmask", [128, 3], BF16, kind="ExternalInput")
    out_d = nc.dram_tensor("out", [TH, D], BF16, kind="ExternalOutput")

    with tile.TileContext(nc) as tc, ExitStack() as ctx:
        const = ctx.enter_context(tc.tile_pool(name="const", bufs=1))
        big = ctx.enter_context(tc.tile_pool(name="big", bufs=1))
        wdram = ctx.enter_context(tc.tile_pool(name="wdram", bufs=1, space="DRAM"))

        # warm up the CC stream so the real S/Z collective has low latency
        warm_in = wdram.tile([1, 8], F32)
        warm_out = wdram.tile([1, 8], F32)
        nc.gpsimd.collective_compute(
            "AllReduce",
            ALU.add,
            replica_groups=[[0, 4], [1, 5], [2, 6], [3, 7]],
            ins=[warm_in[:].opt()],
            outs=[warm_out[:].opt()],
        )

        kt_nat = big.tile([128, 4, KV], BF16)
        vt_nat = big.tile([128, 4, KV], BF16)
        qt_nat = big.tile([128, 4, TH], BF16)
        et_big = big.tile([128, 2 * NIT, 128], FP8)

        # phase-1 operands first so the PE can start immediately.
        a, b_ = KV_GROUPS[0]
        nc.sync.dma_start(
            kt_nat[:, :, a:b_],
            kt_d[:, 4 * a : 4 * b_].rearrange("p (c n) -> p c n", c=4),
        )
        wk_sb = const.tile([128, 4, D], BF16, tag="wk")
        nc.sync.dma_start(
            wk_sb[:], wk_d[:, :].rearrange("p (c n) -> p c n", c=4)
        )
        wv_sb = const.tile([128, 4, D], BF16, tag="wv")
        nc.sync.dma_start(
            wv_sb[:], wv_d[:, :].rearrange("p (c n) -> p c n", c=4)
        )
        nc.sync.dma_start(
            vt_nat[:, :, a:b_],
            vt_d[:, 4 * a : 4 * b_].rearrange("p (c n) -> p c n", c=4),
        )
        szm = const.tile([128, 3], BF16)
        nc.scalar.dma_start(szm[:], szm_d[:, :])
        bqt = const.tile([128, 4], F32)
        nc.scalar.dma_start(bqt[:], bqt_d[:, :])
        bvt = const.tile([128, 4], F32)
        nc.scalar.dma_start(bvt[:], bvt_d[:, :])
        for a, b_ in KV_GROUPS[1:]:
            nc.sync.dma_start(
                kt_nat[:, :, a:b_],
                kt_d[:, 4 * a : 4 * b_].rearrange("p (c n) -> p c n", c=4),
            )
            nc.sync.dma_start(
                vt_nat[:, :, a:b_],
                vt_d[:, 4 * a : 4 * b_].rearrange("p (c n) -> p c n", c=4),
            )

        wq_sb = const.tile([128, 4, D], BF16, tag="wq")
        nc.sync.dma_start(
            wq_sb[:], wq_d[:, :].rearrange("p (c n) -> p c n", c=4)
        )
        nc.sync.dma_start(
            et_big[:], et_d[:, :].rearrange("p (c n) -> p c n", c=2 * NIT)
        )
        for a, b_ in QT_GROUPS:
            nc.sync.dma_start(
                qt_nat[:, :, a:b_],
                qt_d[:, 4 * a : 4 * b_].rearrange("p (c n) -> p c n", c=4),
            )
        wo_sb = const.tile([128, 4, D], BF16, tag="wo")
        nc.sync.dma_start(
            wo_sb[:], wo_d[:, :].rearrange("p (c n) -> p c n", c=4)
        )

        ln8b = const.tile([128, 1], F32)
        nc.vector.memset(ln8b[:], NEG_LN8)
        ek_big = big.tile([128, KCH, D], BF16)
        ekv_big = big.tile([128, KCH, D], BF16)
        ek8_big = big.tile([128, KCH, D], FP8)
        ekv8_big = big.tile([128, KCH, D], FP8)
        sigt = big.tile([128, 4, TH], BF16)

        work = ctx.enter_context(tc.tile_pool(name="work", bufs=3))
        dram = ctx.enter_context(tc.tile_pool(name="dram", bufs=1, space="DRAM"))

        pj_ctx = ExitStack()
        pj_ps = pj_ctx.enter_context(tc.tile_pool(name="pjps", bufs=3, space="PSUM"))

        # ---- phase 1: k/v projections, ek/ekv, S/Z (s lags one chunk) ----
        with tc.tile_pool(name="szps", bufs=1, space="PSUM") as szp:
            z_ps = szp.tile([1, D], F32, tag="z")
            s_ps = szp.tile([1, D], F32, tag="s")

            def s_mm(c):
                sel = 0 if c == 0 else (2 if c == KCH - 1 else 1)
                nc.tensor.matmul(
                    s_ps[:], szm[:, sel : sel + 1], ekv_big[:, c, :],
                    start=(c == 0), stop=(c == KCH - 1), skip_group_check=True,
                )

            for c in range(KCH):
                sel = 0 if c == 0 else (2 if c == KCH - 1 else 1)
                kp = pj_ps.tile([128, D], F32, tag="pj")
                for j in range(4):
                    nc.tensor.matmul(
                        kp[:], kt_nat[:, j, ts(c, 128)], wk_sb[:, j, :],
                        start=(j == 0), stop=(j == 3),
                    )
                nc.scalar.activation(ek_big[:, c, :], kp[:], AF.Exp)
                nc.scalar.activation(
                    ek8_big[:, c, :], kp[:], AF.Exp, bias=ln8b[:, 0:1]
                )

                vp = pj_ps.tile([128, D], F32, tag="pj")
                for j in range(4):
                    nc.tensor.matmul(
                        vp[:], vt_nat[:, j, ts(c, 128)], wv_sb[:, j, :],
                        start=(j == 0), stop=(j == 3),
                    )
                nc.vector.tensor_mul(ekv_big[:, c, :], ek_big[:, c, :], vp[:])
                nc.vector.tensor_scalar_mul(
                    ekv8_big[:, c, :], ekv_big[:, c, :], 0.125
                )

                nc.tensor.matmul(
                    z_ps[:], szm[:, sel : sel + 1], ek_big[:, c, :],
                    start=(c == 0), stop=(c == KCH - 1), skip_group_check=True,
                )
                if c > 0:
                    s_mm(c - 1)
            s_mm(KCH - 1)

            sz_sb = const.tile([1, 2 * D], F32)
            nc.vector.tensor_copy(sz_sb[:, 0:D], z_ps[:])
            nc.vector.tensor_copy(sz_sb[:, D : 2 * D], s_ps[:])

        # ---- phase 2: S/Z all-reduce over the core pair ----
        cc_in = dram.tile([1, 2 * D], F32)
        cc_out = dram.tile([1, 2 * D], F32)
        nc.gpsimd.dma_start(cc_in[:], sz_sb[:])
        nc.gpsimd.collective_compute(
            "AllReduce",
            ALU.add,
            replica_groups=[[0, 4], [1, 5], [2, 6], [3, 7]],
            ins=[cc_in[:].opt()],
            outs=[cc_out[:].opt()],
        )
        # diagonal re-read: zs2[p, t, b] = cc_out[0, t*512 + b*128 + p],
        # so Z/S become per-partition scalars for the transposed combine.
        zs2 = const.tile([128, 2, 4], F32)  # [:,0,b]=Z block b, [:,1,b]=S
        nc.gpsimd.dma_start(
            zs2[:], cc_out[0:1, :].rearrange("r (t b p) -> p (r t) b", t=2, b=4, p=128)
        )

        # ---- phase 4a: q^T projections; sigmoid(q^T + bq) via act bias ----
        for b in range(4):
            for g in range(4):
                qp = pj_ps.tile([128, D], F32, tag="pj")
                for j in range(4):
                    nc.tensor.matmul(
                        qp[:], wq_sb[:, j, ts(b, 128)],
                        qt_nat[:, j, ts(g, 512)],
                        start=(j == 0), stop=(j == 3),
                    )
                nc.scalar.activation(
                    sigt[:, b, ts(g, 512)], qp[:], AF.Sigmoid,
                    bias=bqt[:, b : b + 1],
                )

        # ---- phase 4b: band einsum, combine, out proj (pipelined) ----
        pj_ctx.close()  # release projection PSUM banks
        ein_ps = ctx.enter_context(tc.tile_pool(name="einps", bufs=3, space="PSUM"))
        op_ps = ctx.enter_context(tc.tile_pool(name="opps", bufs=2, space="PSUM"))
        bvfull = const.tile([128, 4, 128], BF16)
        nc.vector.tensor_copy(bvfull[:], bvt[:, :].broadcast_to([128, 4, 128]))

        def einsum(it):
            bn = ein_ps.tile([128, 4, 128], F32, tag="bn")
            bd = ein_ps.tile([128, 4, 128], F32, tag="bd")
            et = et_big[:, 2 * it : 2 * it + 2, :]
            for b in range(4):
                nc.tensor.matmul(
                    bd[:, b, :], ek8_big[:, it : it + 2, ts(b, 128)], et,
                    start=True, stop=True, perf_mode=DR,
                )
                nc.tensor.matmul(
                    bn[:, b, :], ekv8_big[:, it : it + 2, ts(b, 128)], et,
                    start=True, stop=True, perf_mode=DR,
                )
            return bn, bd

        def combine(it, bn, bd):
            # scalar: den_b = bd_b + Z_b (f32, feeds the DVE reciprocal)
            den = work.tile([128, 4, 128], F32, tag="den")
            for b in range(4):
                nc.scalar.activation(
                    den[:, b, :], bd[:, b, :], AF.Identity,
                    bias=zs2[:, 0, b : b + 1],
                )
            # DVE: rec = 1/den; y1_b = (bn_b + S_b) * rec_b (fused stt)
            rec = work.tile([128, 4, 128], F32, tag="rec")
            nc.vector.reciprocal_approx_fast(rec[:], den[:])
            y1 = work.tile([128, 4, 128], BF16, tag="y1")
            for b in range(4):
                nc.vector.scalar_tensor_tensor(
                    y1[:, b, :], bn[:, b, :], zs2[:, 1, b : b + 1],
                    rec[:, b, :], ALU.add, ALU.mult,
                )
            # Pool: y2 = y1 + bv; DVE (2x bf16): yts = y2 * sig
            y2 = work.tile([128, 4, 128], BF16, tag="y2")
            nc.gpsimd.tensor_tensor(y2[:], y1[:], bvfull[:], ALU.add)
            yts = work.tile([128, 4, 128], BF16, tag="yts")
            nc.vector.tensor_tensor(
                yts[:], y2[:], sigt[:, :, ts(it, 128)], ALU.mult
            )
            return yts

        def oproj_mm(it, yts):
            op = op_ps.tile([128, D], F32, tag="op")
            for j in range(4):
                nc.tensor.matmul(
                    op[:], yts[:, j, :], wo_sb[:, j, :],
                    start=(j == 0), stop=(j == 3),
                )
            return op

        otp = ctx.enter_context(tc.tile_pool(name="otp", bufs=2))

        def ot_out(it, op):
            if it >= NIT - 4:
                ot = work.tile([128, D], BF16, tag="ot")
                nc.scalar.activation(ot[:], op[:], AF.Copy)
                nc.sync.dma_start(out_d[ts(it, 128), :], ot[:])
                return
            g, r = it // 4, it % 4
            if r == 0:
                ot_out.cur = otp.tile([128, 4, D], BF16, tag="otg")
            nc.scalar.activation(ot_out.cur[:, r, :], op[:], AF.Copy)
            if r == 3:
                nc.sync.dma_start(
                    out_d[ts(g, 512), :].rearrange("(c p) n -> p c n", p=128),
                    ot_out.cur[:],
                )

        ein_q = []
        for step in range(NIT + 3):
            it = op = None
            if step >= 3:
                it, bn, bd = ein_q.pop(0)
                op = oproj_mm(it, combine(it, bn, bd))
            if step < NIT:
                ein_q.append((step, *einsum(step)))
            if op is not None:
                ot_out(it, op)

    nc.finalize()
    return nc


def _shard_inputs(inputs):
    query = np.asarray(inputs["query"], np.float32)
    key = np.asarray(inputs["key"], np.float32)
    value = np.asarray(inputs["value"], np.float32)
    pos_bias = np.asarray(inputs["pos_bias"], np.float32)

    def pack_w(w):
        # [D, D] W.T -> [128, 4*D] with row d_in = c*128+p at [p, c*D:(c+1)*D]
        wt = np.ascontiguousarray(np.asarray(w, np.float32).T.astype(BFNP))
        return np.ascontiguousarray(
            wt.reshape(4, 128, D).transpose(1, 0, 2).reshape(128, 4 * D)
        )

    def pack_x(xt, groups):
        # xt [D, N] -> group-concatenated [128, 4*N]
        a4 = xt.reshape(4, 128, xt.shape[1]).transpose(1, 0, 2)  # [p, c, n]
        blocks = [
            np.ascontiguousarray(a4[:, :, a:b]).reshape(128, -1)
            for a, b in groups
        ]
        return np.ascontiguousarray(np.concatenate(blocks, axis=1))

    wq = pack_w(inputs["Wq"])
    wo = pack_w(inputs["Wo"])
    wk = pack_w(inputs["Wk"])
    wv = pack_w(inputs["Wv"])
    bqt = np.ascontiguousarray(
        np.asarray(inputs["bq"], np.float32).reshape(4, 128).T
    )
    bvt = np.ascontiguousarray(
        np.asarray(inputs["bv"], np.float32).reshape(4, 128).T
    )
    # note: bk cancels in num/den and is deliberately unused.

    # band mask in transposed block coords: block row jj (j = i0-64+jj),
    # col ii; nonzero iff |i - j| < 64  <=>  ii < jl < ii + 128.
    jl = np.arange(256)[:, None]
    il = np.arange(128)[None, :]
    bandT = ((jl > il) & (jl < il + 128)).astype(np.float32)

    szmask = np.zeros((128, 3), np.float32)
    szmask[64:, 0] = 1.0
    szmask[:, 1] = 1.0
    szmask[:64, 2] = 1.0

    in_maps = []
    for c in range(8):
        b, h = c % 4, c // 4
        r0 = h * TH
        qt_s = pack_x(query[r0 : r0 + TH, b, :].T.astype(BFNP), QT_GROUPS)

        lo, hi = r0 - WIN, r0 + TH + WIN
        kt_t = np.zeros((D, KV), BFNP)
        vt_t = np.zeros((D, KV), BFNP)
        slo, shi = max(lo, 0), min(hi, T)
        kt_t[:, slo - lo : shi - lo] = key[slo:shi, b, :].T.astype(BFNP)
        vt_t[:, slo - lo : shi - lo] = value[slo:shi, b, :].T.astype(BFNP)
        kt_s = pack_x(kt_t, KV_GROUPS)
        vt_s = pack_x(vt_t, KV_GROUPS)

        # E^T = exp(pb^T) - 1, banded, in [j, i] blocks (host-precomputed)
        et_s = np.zeros((NIT, 256, 128), np.float32)
        for it in range(NIT):
            i0 = r0 + it * 128
            j0 = i0 - WIN
            js, je = max(j0, 0), min(j0 + 256, T)
            et_s[it, js - j0 : je - j0, :] = pos_bias[i0 : i0 + 128, js:je].T
        et_s = 8.0 * np.expm1(et_s) * bandT[None]
        # [NIT, 2, 128(p), 128(i)] -> [128, NIT*2*128]
        et_h = np.ascontiguousarray(
            et_s.reshape(NIT, 2, 128, 128)
            .transpose(2, 0, 1, 3)
            .reshape(128, 2 * NIT * 128)
            .astype(F8NP)
        )

        m = {
            "qt": qt_s,
            "kt": kt_s,
            "vt": vt_s,
            "et": et_h,
            "bqt": bqt,
            "bvt": bvt,
            "szmask": szmask.astype(BFNP),
            "wq": wq,
            "wo": wo,
            "wk": wk,
            "wv": wv,
        }
        in_maps.append(m)
    return in_maps


def run(trace=False, **inputs):
    in_maps = _shard_inputs(inputs)
    nc = build()
    res = run_bass_kernel_spmd(nc, in_maps, core_ids=list(range(8)), trace=trace)
    out = np.zeros((T, B, D), np.float32)
    bo = np.asarray(inputs["bo"], np.float32).reshape(1, D)
    for c in range(8):
        b, h = c % 4, c // 4
        out[h * TH : (h + 1) * TH, b, :] = (
            np.asarray(res.results[c]["out"], np.float32) + bo
        )
    return out, res


def kernel(**inputs) -> np.ndarray:
    out, _ = run(trace=False, **inputs)
    return out


# revision 17
# speedup vs baseline: 1.0796x; 1.0796x over previous
"""AFT local-attention kernel for Trainium2 (8 NeuronCores), v4.

Math (identical to reference in exact arithmetic):
  y = sigmoid(q) * num / den, out = y @ Wo.T + bo, with
    E[i,j] = (exp(pb[i,j]) - 1) * band_mask[i,j]   (band width 127)
    ek     = exp(key @ Wk.T)                       (bk and the exp-max
                                                    stabilizers cancel)
    den    = E @ ek + Z,  Z[b,d] = sum_j ek[j,b,d]  (own rows only)
    num'   = E @ (ek*v') + S', v' = value @ Wv.T    (no bv)
    num/den = num'/den + bv                         (bv folds out)

v4 structure:
  - All inputs host-packed into device-native [128, ...] layouts so every
    DMA line is contiguous per partition (k/v/q split into column groups
    for pipelining).
  - A tiny warmup AllGather at kernel start pre-builds the CC stream so
    the real S/Z pair-exchange has ~8us latency instead of ~26us.
  - Phase order: k/v proj + S/Z -> trigger collective -> q proj (overlaps
    collective) -> einsum/combine/out-proj pipeline (zs2 arrives before
    the first combine needs it).
  - Combine work split across three engines: scalar (den bias-acts + ot
    copy), DVE (num add, reciprocal, y1), Pool (y2 + bv, yts * sig).
  - Output written in bf16; host casts to f32 and adds bo.

Sharding: core c -> batch b=c//2, T-half h=c%2 (2048 queries each); k/v
carry a 64-row halo; S/Z are all-gathered over core pairs [2b, 2b+1].
"""

import numpy as np
from contextlib import ExitStack

import ml_dtypes
import concourse.bass as bass
import concourse.tile as tile
from concourse import bacc, mybir
from concourse.bass import ts, ds
from concourse.bass_utils import run_bass_kernel_spmd

T = 4096
B = 4
D = 512
WIN = 64
TH = T // 2          # tokens per core (query rows)
NIT = TH // 128      # 16 i-tiles
KV = TH + 2 * WIN    # 2176 k/v rows incl. halo
KCH = KV // 128      # 17 k/v chunks

F32 = mybir.dt.float32
BF16 = mybir.dt.bfloat16
FP8 = mybir.dt.float8e4
AF = mybir.ActivationFunctionType
ALU = mybir.AluOpType
DR = mybir.MatmulPerfMode.DoubleRow
BFNP = ml_dtypes.bfloat16
F8NP = ml_dtypes.float8_e4m3
NEG_LN8 = -2.0794415416798357

KV_GROUPS = [(0, 256), (256, 768), (768, 1280), (1280, 1792), (1792, KV)]
QT_GROUPS = [(0, 1024), (1024, TH)]


def build():
    nc = bacc.Bacc("TRN2", target_bir_lowering=False, num_devices=8)

    # flat host-packed inputs; per-partition lines are contiguous
    kt_d = nc.dram_tensor("kt", [128, 4 * KV], BF16, kind="ExternalInput")
    vt_d = nc.dram_tensor("vt", [128, 4 * KV], BF16, kind="ExternalInput")
    qt_d = nc.dram_tensor("qt", [128, 4 * TH], BF16, kind="ExternalInput")
    et_d = nc.dram_tensor("et", [128, 2 * NIT * 128], FP8, kind="ExternalInput")
    wq_d = nc.dram_tensor("wq", [128, 4 * D], BF16, kind="ExternalInput")
    wo_d = nc.dram_tensor("wo", [128, 4 * D], BF16, kind="ExternalInput")
    wk_d = nc.dram_tensor("wk", [128, 4 * D], BF16, kind="ExternalInput")
    wv_d = nc.dram_tensor("wv", [128, 4 * D], BF16, kind="ExternalInput")
    bqt_d = nc.dram_tensor("bqt", [128, 4], F32, kind="ExternalInput")
    bvt_d = nc.dram_tensor("bvt", [128, 4], F32, kind="ExternalInput")
    szm_d = nc.dram_tensor("szmask", [128, 3], BF16, kind="ExternalInput")
    out_d = nc.dram_tensor("out", [TH, D], BF16, kind="ExternalOutput")

    with tile.TileContext(nc) as tc, ExitStack() as ctx:
        const = ctx.enter_context(tc.tile_pool(name="const", bufs=1))
        big = ctx.enter_context(tc.tile_pool(name="big", bufs=1))
        wdram = ctx.enter_context(tc.tile_pool(name="wdram", bufs=1, space="DRAM"))

        # warm up the CC stream so the real S/Z collective has low latency
        warm_in = wdram.tile([1, 8], F32)
        warm_out = wdram.tile([1, 8], F32)
        nc.gpsimd.collective_compute(
            "AllReduce",
            ALU.add,
            replica_groups=[[0, 1], [2, 3], [4, 5], [6, 7]],
            ins=[warm_in[:].opt()],
            outs=[warm_out[:].opt()],
        )

        kt_nat = big.tile([128, 4, KV], BF16)
        vt_nat = big.tile([128, 4, KV], BF16)
        qt_nat = big.tile([128, 4, TH], BF16)
        et_big = big.tile([128, 2 * NIT, 128], FP8)

        # phase-1 operands first so the PE can start immediately.
        a, b_ = KV_GROUPS[0]
        nc.sync.dma_start(
            kt_nat[:, :, a:b_],
            kt_d[:, 4 * a : 4 * b_].rearrange("p (c n) -> p c n", c=4),
        )
        wk_sb = const.tile([128, 4, D], BF16, tag="wk")
        nc.sync.dma_start(
            wk_sb[:], wk_d[:, :].rearrange("p (c n) -> p c n", c=4)
        )
        wv_sb = const.tile([128, 4, D], BF16, tag="wv")
        nc.sync.dma_start(
            wv_sb[:], wv_d[:, :].rearrange("p (c n) -> p c n", c=4)
        )
        nc.sync.dma_start(
            vt_nat[:, :, a:b_],
            vt_d[:, 4 * a : 4 * b_].rearrange("p (c n) -> p c n", c=4),
        )
        szm = const.tile([128, 3], BF16)
        nc.scalar.dma_start(szm[:], szm_d[:, :])
        bqt = const.tile([128, 4], F32)
        nc.scalar.dma_start(bqt[:], bqt_d[:, :])
        bvt = const.tile([128, 4], F32)
        nc.scalar.dma_start(bvt[:], bvt_d[:, :])
        for a, b_ in KV_GROUPS[1:]:
            nc.sync.dma_start(
                kt_nat[:, :, a:b_],
                kt_d[:, 4 * a : 4 * b_].rearrange("p (c n) -> p c n", c=4),
            )
            nc.sync.dma_start(
                vt_nat[:, :, a:b_],
                vt_d[:, 4 * a : 4 * b_].rearrange("p (c n) -> p c n", c=4),
            )

        wq_sb = const.tile([128, 4, D], BF16, tag="wq")
        nc.sync.dma_start(
            wq_sb[:], wq_d[:, :].rearrange("p (c n) -> p c n", c=4)
        )
        nc.sync.dma_start(
            et_big[:], et_d[:, :].rearrange("p (c n) -> p c n", c=2 * NIT)
        )
        for a, b_ in QT_GROUPS:
            nc.sync.dma_start(
                qt_nat[:, :, a:b_],
                qt_d[:, 4 * a : 4 * b_].rearrange("p (c n) -> p c n", c=4),
            )
        wo_sb = const.tile([128, 4, D], BF16, tag="wo")
        nc.sync.dma_start(
            wo_sb[:], wo_d[:, :].rearrange("p (c n) -> p c n", c=4)
        )

        ln8b = const.tile([128, 1], F32)
        nc.vector.memset(ln8b[:], NEG_LN8)
        ek_big = big.tile([128, KCH, D], BF16)
        ekv_big = big.tile([128, KCH, D], BF16)
        ek8_big = big.tile([128, KCH, D], FP8)
        ekv8_big = big.tile([128, KCH, D], FP8)
        sigt = big.tile([128, 4, TH], BF16)

        work = ctx.enter_context(tc.tile_pool(name="work", bufs=3))
        dram = ctx.enter_context(tc.tile_pool(name="dram", bufs=1, space="DRAM"))

        pj_ctx = ExitStack()
        pj_ps = pj_ctx.enter_context(tc.tile_pool(name="pjps", bufs=3, space="PSUM"))

        # ---- phase 1: k/v projections, ek/ekv, S/Z (s lags one chunk) ----
        with tc.tile_pool(name="szps", bufs=1, space="PSUM") as szp:
            z_ps = szp.tile([1, D], F32, tag="z")
            s_ps = szp.tile([1, D], F32, tag="s")

            def s_mm(c):
                sel = 0 if c == 0 else (2 if c == KCH - 1 else 1)
                nc.tensor.matmul(
                    s_ps[:], szm[:, sel : sel + 1], ekv_big[:, c, :],
                    start=(c == 0), stop=(c == KCH - 1), skip_group_check=True,
                )

            for c in range(KCH):
                sel = 0 if c == 0 else (2 if c == KCH - 1 else 1)
                kp = pj_ps.tile([128, D], F32, tag="pj")
                for j in range(4):
                    nc.tensor.matmul(
                        kp[:], kt_nat[:, j, ts(c, 128)], wk_sb[:, j, :],
                        start=(j == 0), stop=(j == 3),
                    )
                nc.scalar.activation(ek_big[:, c, :], kp[:], AF.Exp)
                nc.scalar.activation(
                    ek8_big[:, c, :], kp[:], AF.Exp, bias=ln8b[:, 0:1]
                )

                vp = pj_ps.tile([128, D], F32, tag="pj")
                for j in range(4):
                    nc.tensor.matmul(
                        vp[:], vt_nat[:, j, ts(c, 128)], wv_sb[:, j, :],
                        start=(j == 0), stop=(j == 3),
                    )
                nc.vector.tensor_mul(ekv_big[:, c, :], ek_big[:, c, :], vp[:])
                nc.vector.tensor_scalar_mul(
                    ekv8_big[:, c, :], ekv_big[:, c, :], 0.125
                )

                nc.tensor.matmul(
                    z_ps[:], szm[:, sel : sel + 1], ek_big[:, c, :],
                    start=(c == 0), stop=(c == KCH - 1), skip_group_check=True,
                )
                if c > 0:
                    s_mm(c - 1)
            s_mm(KCH - 1)

            sz_sb = const.tile([1, 2 * D], F32)
            nc.vector.tensor_copy(sz_sb[:, 0:D], z_ps[:])
            nc.vector.tensor_copy(sz_sb[:, D : 2 * D], s_ps[:])

        # ---- phase 2: S/Z all-reduce over the core pair ----
        cc_in = dram.tile([1, 2 * D], F32)
        cc_out = dram.tile([1, 2 * D], F32)
        nc.gpsimd.dma_start(cc_in[:], sz_sb[:])
        nc.gpsimd.collective_compute(
            "AllReduce",
            ALU.add,
            replica_groups=[[0, 1], [2, 3], [4, 5], [6, 7]],
            ins=[cc_in[:].opt()],
            outs=[cc_out[:].opt()],
        )
        # diagonal re-read: zs2[p, t, b] = cc_out[0, t*512 + b*128 + p],
        # so Z/S become per-partition scalars for the transposed combine.
        zs2 = const.tile([128, 2, 4], F32)  # [:,0,b]=Z block b, [:,1,b]=S
        nc.gpsimd.dma_start(
            zs2[:], cc_out[0:1, :].rearrange("r (t b p) -> p (r t) b", t=2, b=4, p=128)
        )

        # ---- phase 4a: q^T projections; sigmoid(q^T + bq) via act bias ----
        for b in range(4):
            for g in range(4):
                qp = pj_ps.tile([128, D], F32, tag="pj")
                for j in range(4):
                    nc.tensor.matmul(
                        qp[:], wq_sb[:, j, ts(b, 128)],
                        qt_nat[:, j, ts(g, 512)],
                        start=(j == 0), stop=(j == 3),
                    )
                nc.scalar.activation(
                    sigt[:, b, ts(g, 512)], qp[:], AF.Sigmoid,
                    bias=bqt[:, b : b + 1],
                )

        # ---- phase 4b: band einsum, combine, out proj (pipelined) ----
        pj_ctx.close()  # release projection PSUM banks
        ein_ps = ctx.enter_context(tc.tile_pool(name="einps", bufs=3, space="PSUM"))
        op_ps = ctx.enter_context(tc.tile_pool(name="opps", bufs=2, space="PSUM"))
        bvfull = const.tile([128, 4, 128], BF16)
        nc.vector.tensor_copy(bvfull[:], bvt[:, :].broadcast_to([128, 4, 128]))

        def einsum(it):
            bn = ein_ps.tile([128, 4, 128], F32, tag="bn")
            bd = ein_ps.tile([128, 4, 128], F32, tag="bd")
            et = et_big[:, 2 * it : 2 * it + 2, :]
            for b in range(4):
                nc.tensor.matmul(
                    bd[:, b, :], ek8_big[:, it : it + 2, ts(b, 128)], et,
                    start=True, stop=True, perf_mode=DR,
                )
                nc.tensor.matmul(
                    bn[:, b, :], ekv8_big[:, it : it + 2, ts(b, 128)], et,
                    start=True, stop=True, perf_mode=DR,
                )
            return bn, bd

        def combine(it, bn, bd):
            # scalar: den_b = bd_b + Z_b (f32, feeds the DVE reciprocal)
            den = work.tile([128, 4, 128], F32, tag="den")
            for b in range(4):
                nc.scalar.activation(
                    den[:, b, :], bd[:, b, :], AF.Identity,
                    bias=zs2[:, 0, b : b + 1],
                )
            # DVE: rec = 1/den; y1_b = (bn_b + S_b) * rec_b (fused stt)
            rec = work.tile([128, 4, 128], F32, tag="rec")
            nc.vector.reciprocal_approx_fast(rec[:], den[:])
            y1 = work.tile([128, 4, 128], BF16, tag="y1")
            for b in range(4):
                nc.vector.scalar_tensor_tensor(
                    y1[:, b, :], bn[:, b, :], zs2[:, 1, b : b + 1],
                    rec[:, b, :], ALU.add, ALU.mult,
                )
            # Pool: y2 = y1 + bv; yts split pool/DVE to balance engines
            y2 = work.tile([128, 4, 128], BF16, tag="y2")
            nc.gpsimd.tensor_tensor(y2[:], y1[:], bvfull[:], ALU.add)
            yts = work.tile([128, 4, 128], BF16, tag="yts")
            nc.gpsimd.tensor_tensor(
                yts[:, 0:2], y2[:, 0:2], sigt[:, 0:2, ts(it, 128)], ALU.mult
            )
            nc.vector.tensor_tensor(
                yts[:, 2:4], y2[:, 2:4], sigt[:, 2:4, ts(it, 128)], ALU.mult
            )
            return yts

        def oproj_mm(it, yts):
            op = op_ps.tile([128, D], F32, tag="op")
            for j in range(4):
                nc.tensor.matmul(
                    op[:], yts[:, j, :], wo_sb[:, j, :],
                    start=(j == 0), stop=(j == 3),
                )
            return op

        otp = ctx.enter_context(tc.tile_pool(name="otp", bufs=2))

        def ot_out(it, op):
            if it >= NIT - 4:
                ot = work.tile([128, D], BF16, tag="ot")
                nc.scalar.activation(ot[:], op[:], AF.Copy)
                nc.sync.dma_start(out_d[ts(it, 128), :], ot[:])
                return
            g, r = it // 4, it % 4
            if r == 0:
                ot_out.cur = otp.tile([128, 4, D], BF16, tag="otg")
            nc.scalar.activation(ot_out.cur[:, r, :], op[:], AF.Copy)
            if r == 3:
                nc.sync.dma_start(
                    out_d[ts(g, 512), :].rearrange("(c p) n -> p c n", p=128),
                    ot_out.cur[:],
                )

        ein_q = []
        for step in range(NIT + 3):
            it = op = None
            if step >= 3:
                it, bn, bd = ein_q.pop(0)
                op = oproj_mm(it, combine(it, bn, bd))
            if step < NIT:
                ein_q.append((step, *einsum(step)))
            if op is not None:
                ot_out(it, op)

    nc.finalize()
    return nc


def _shard_inputs(inputs):
    query = np.asarray(inputs["query"], np.float32)
    key = np.asarray(inputs["key"], np.float32)
    value = np.asarray(inputs["value"], np.float32)
    pos_bias = np.asarray(inputs["pos_bias"], np.float32)

    def pack_w(w):
        # [D, D] W.T -> [128, 4*D] with row d_in = c*128+p at [p, c*D:(c+1)*D]
        wt = np.ascontiguousarray(np.asarray(w, np.float32).T.astype(BFNP))
        return np.ascontiguousarray(
            wt.reshape(4, 128, D).transpose(1, 0, 2).reshape(128, 4 * D)
        )

    def pack_x(xt, groups):
        # xt [D, N] -> group-concatenated [128, 4*N]
        a4 = xt.reshape(4, 128, xt.shape[1]).transpose(1, 0, 2)  # [p, c, n]
        blocks = [
            np.ascontiguousarray(a4[:, :, a:b]).reshape(128, -1)
            for a, b in groups
        ]
        return np.ascontiguousarray(np.concatenate(blocks, axis=1))

    wq = pack_w(inputs["Wq"])
    wo = pack_w(inputs["Wo"])
    wk = pack_w(inputs["Wk"])
    wv = pack_w(inputs["Wv"])
    bqt = np.ascontiguousarray(
        np.asarray(inputs["bq"], np.float32).reshape(4, 128).T
    )
    bvt = np.ascontiguousarray(
        np.asarray(inputs["bv"], np.float32).reshape(4, 128).T
    )
    # note: bk cancels in num/den and is deliberately unused.

    # band mask in transposed block coords: block row jj (j = i0-64+jj),
    # col ii; nonzero iff |i - j| < 64  <=>  ii < jl < ii + 128.
    jl = np.arange(256)[:, None]
    il = np.arange(128)[None, :]
    bandT = ((jl > il) & (jl < il + 128)).astype(np.float32)

    szmask = np.zeros((128, 3), np.float32)
    szmask[64:, 0] = 1.0
    szmask[:, 1] = 1.0
    szmask[:64, 2] = 1.0

    in_maps = []
    for c in range(8):
        b, h = c // 2, c % 2
        r0 = h * TH
        qt_s = pack_x(query[r0 : r0 + TH, b, :].T.astype(BFNP), QT_GROUPS)

        lo, hi = r0 - WIN, r0 + TH + WIN
        kt_t = np.zeros((D, KV), BFNP)
        vt_t = np.zeros((D, KV), BFNP)
        slo, shi = max(lo, 0), min(hi, T)
        kt_t[:, slo - lo : shi - lo] = key[slo:shi, b, :].T.astype(BFNP)
        vt_t[:, slo - lo : shi - lo] = value[slo:shi, b, :].T.astype(BFNP)
        kt_s = pack_x(kt_t, KV_GROUPS)
        vt_s = pack_x(vt_t, KV_GROUPS)

        # E^T = exp(pb^T) - 1, banded, in [j, i] blocks (host-precomputed)
        et_s = np.zeros((NIT, 256, 128), np.float32)
        for it in range(NIT):
            i0 = r0 + it * 128
            j0 = i0 - WIN
            js, je = max(j0, 0), min(j0 + 256, T)
            et_s[it, js - j0 : je - j0, :] = pos_bias[i0 : i0 + 128, js:je].T
        et_s = 8.0 * np.expm1(et_s) * bandT[None]
        # [NIT, 2, 128(p), 128(i)] -> [128, NIT*2*128]
        et_h = np.ascontiguousarray(
            et_s.reshape(NIT, 2, 128, 128)
            .transpose(2, 0, 1, 3)
            .reshape(128, 2 * NIT * 128)
            .astype(F8NP)
        )

        m = {
            "qt": qt_s,
            "kt": kt_s,
            "vt": vt_s,
            "et": et_h,
            "bqt": bqt,
            "bvt": bvt,
            "szmask": szmask.astype(BFNP),
            "wq": wq,
            "wo": wo,
            "wk": wk,
            "wv": wv,
        }
        in_maps.append(m)
    return in_maps


def run(trace=False, **inputs):
    in_maps = _shard_inputs(inputs)
    nc = build()
    res = run_bass_kernel_spmd(nc, in_maps, core_ids=list(range(8)), trace=trace)
    out = np.zeros((T, B, D), np.float32)
    bo = np.asarray(inputs["bo"], np.float32).reshape(1, D)
    for c in range(8):
        b, h = c // 2, c % 2
        out[h * TH : (h + 1) * TH, b, :] = (
            np.asarray(res.results[c]["out"], np.float32) + bo
        )
    return out, res


def kernel(**inputs) -> np.ndarray:
    out, _ = run(trace=False, **inputs)
    return out
